# revision 22
# baseline (speedup 1.0000x reference)
"""AttentiveChildSumTreeLSTMCell on 8 Trainium2 NeuronCores.

Tensor-parallel: column-parallel f/attention/iou linears (hidden dim sharded
8 ways), row-parallel merge linear.  Cross-core exchanges: a warmup collective
triggered immediately at kernel start (absorbs the ~60us first-collective
init barrier), AllGather of partial attention logits, AllReduce of
merge-linear partials + f LayerNorm stats, AllGather of iou/forget*cell
chunks.

Perf notes vs the naive schedule:
- All inputs are packed host-side into 9 large DMAs (one bf16 activation
  pack, two fp32 constant packs, five weight streams) to kill the ~0.65us
  per-DMA dispatch serialization on the Sync queue.
- Every activation uses the sigmoid_and_others ACT table set (sigmoid+tanh):
  softmax exp is computed as (1+t)/(1-t) with t=tanh(x/2); LayerNorm rsqrt
  for the f gate uses a Quake-style bit-trick seed + 2 Newton steps on DVE.
  One table load for the whole kernel, zero mid-kernel ACT_TABLE_LOADs.
- The four full-width (2048) LayerNorms (merge, i, o, u, cell) run as single
  fused InstLayernorm instructions on GpSimd.
- AG3 results are read back with rearranged-AP DMAs directly into t-major
  [128,16] tiles (no one-hot selector matmuls).
- Dummy matmuls during the AR2 window keep the PE HAM clock warm so the
  post-AR2 iou matmuls stream at 2.4GHz instead of 1.2GHz.
"""

import sys

for _p in ("/opt/trn_rl_repo",):
    if _p not in sys.path:
        sys.path.insert(0, _p)

import ml_dtypes
import numpy as np

import concourse.bacc as bacc
import concourse.mybir as mybir
import concourse.tile as tile
from concourse.bass_utils import run_bass_kernel_spmd
from concourse.tile_rust import add_dep_helper

F32 = mybir.dt.float32
BF16 = mybir.dt.bfloat16
I32 = mybir.dt.int32
AF = mybir.ActivationFunctionType
ALU = mybir.AluOpType
NPBF = ml_dtypes.bfloat16

H = 2048
N = 32
NC = 8
S = H // NC           # 256: per-core chunk of every sharded dim
T = H // 128          # 16 tiles of 128 along a 2048 dim
KT = 32               # K-tiles along the 4096 contraction dims
EPS = 1e-5
INV_H = 1.0 / H
MAGIC = 0x5F3759DF    # rsqrt bit-trick seed constant
WARM_MMS = True      # PE HAM warm-up dummies during the AR2 window

# cpack128 column layout (fp32 [128, CP128])
_LN = ["gm", "bm", "gi", "bi", "go", "bo", "gu", "bu", "gc", "bc"]
C_LN0 = 0             # 10 * T cols of LayerNorm gamma/beta
C_ONE = C_LN0 + 10 * T    # 160: ones column
C_ONESR = C_ONE + 1       # 161..289: row 0 = ones[128] (bcast row)
C_MAGIC = C_ONESR + 128   # 289: rsqrt magic bits
C_SEL = C_MAGIC + 1       # 290..322: sel32 one-hot reassembly (rows 0-7)
CP128 = C_SEL + 32        # 322

_CACHE = {}


def _build(dbg=False):
    nc = bacc.Bacc(None, target_bir_lowering=False, debug=False,
                   num_devices=NC)

    def din(name, shape, dt=F32):
        return nc.dram_tensor(name, list(shape), dt, kind="ExternalInput")

    # ---- per-core DRAM inputs (SPMD: same shapes on every core) ----
    actpack = din("actpack", (128, 1616), BF16)   # hT|xT32|eT32|x1|hTc
    cpack32 = din("cpack32", (32, 1024))          # cells|gf|bf|wattn
    cpack128 = din("cpack128", (128, CP128))      # LN g/b, ones, onesr, magic
    wai = din("wai", (128, KT * S), BF16)         # W_ai^T chunk
    wf = din("wf", (128, KT * S), BF16)           # [W_fh | W_fi]^T chunk
    wmg = din("wmg", (128, 2 * H), BF16)          # W_merge^T in-chunk
    x1d = din("x1d", (128, T), BF16)              # x t-major
    wioux = din("wioux", (128, T * 3 * S), BF16)  # W_iou^T x-half
    wioumh = din("wioumh", (128, T * 3 * S), BF16)  # W_iou^T mh-half

    out_h = nc.dram_tensor("out_h", [128, T], F32, kind="ExternalOutput")
    out_c = nc.dram_tensor("out_c", [128, T], F32, kind="ExternalOutput")
    warm_sink = nc.dram_tensor("warm_sink", [1, 1], F32,
                               kind="ExternalOutput")
    dbg_t = {}
    if dbg:
        for nm, shp in [("d_lg", [N, 1]), ("d_exps", [1, N]),
                        ("d_ml", [128, T]), ("d_mh", [128, T]),
                        ("d_frstd", [N, 1]), ("d_f", [N, S]),
                        ("d_fc", [128, 2]), ("d_iou", [1, 3 * S]),
                        ("d_vec0", [128, 16]), ("d_vec3", [128, 16]),
                        ("d_ioux", [1, 3 * S]), ("d_mhbf", [128, T]),
                        ("d_lni", [128, T]), ("d_cl", [128, T])]:
            dbg_t[nm] = nc.dram_tensor(nm, shp, F32, kind="ExternalOutput")

    with tile.TileContext(nc) as tc:
        with (
            tc.tile_pool(name="sb", bufs=1) as sb,
            tc.tile_pool(name="ps", bufs=1, space="PSUM") as ps,
            tc.tile_pool(name="dram", bufs=1, space="DRAM") as dram,
        ):
            # ------- warmup collective: fires instantly on garbage DRAM ----
            # (content is irrelevant; its only job is to pay the one-time
            # communicator/channel init + cross-core start alignment while
            # the weight DMAs stream)
            warm_in = dram.tile([1, 64], F32, name="warm_in")
            warm_out = dram.tile([8, 64], F32, name="warm_out")
            nc.gpsimd.collective_compute(
                "AllGather", ALU.bypass,
                replica_groups=[list(range(NC))],
                ins=[warm_in.opt()], outs=[warm_out.opt()])

            # ------- ACT table preload: one dummy sigmoid ----
            # all activations in this kernel are sigmoid/tanh -> one set
            tl_scr = sb.tile([1, 1], F32, name="tl_scr")
            nc.vector.memset(tl_scr[:], 0.5)
            nc.scalar.activation(tl_scr[:], tl_scr[:], AF.Sigmoid)

            # ---------------- input DMAs (ordered, 2-deep chained) --------
            act_sb = sb.tile([128, 1616], BF16, name="act_sb")
            x1t_sb = sb.tile([128, T], BF16, name="x1t_sb")
            c32_sb = sb.tile([32, 1024], F32, name="c32_sb")
            c128_sb = sb.tile([128, CP128], F32, name="c128_sb")
            wai_sb = sb.tile([128, KT * S], BF16, name="wai_sb")
            wf_sb = sb.tile([128, KT * S], BF16, name="wf_sb")
            wmg_sb = sb.tile([128, 2 * H], BF16, name="wmg_sb")
            wix_sb = sb.tile([128, T * 3 * S], BF16, name="wix_sb")
            wim_sb = sb.tile([128, T * 3 * S], BF16, name="wim_sb")

            nc.sync.dma_start(x1t_sb[:], x1d[:])
            wdmas = []
            for dst, src in ((act_sb, actpack), (wai_sb, wai),
                             (c32_sb, cpack32), (c128_sb, cpack128),
                             (wf_sb, wf), (wmg_sb, wmg),
                             (wix_sb, wioux), (wim_sb, wioumh)):
                wdmas.append(nc.sync.dma_start(dst[:], src[:]))
            for i in range(2, len(wdmas)):
                add_dep_helper(wdmas[i].ins, wdmas[i - 2].ins, sync=True,
                               reason="weight DMA arrival order")

            # subviews of the packs
            hT_sb = act_sb[:].rearrange(
                "p (x) -> p x")[:, 0:T * N].rearrange(
                "p (t n) -> p t n", t=T)
            xT32_sb = act_sb[:, T * N:2 * T * N].rearrange(
                "p (t n) -> p t n", t=T)
            eT32_sb = act_sb[:, 2 * T * N:3 * T * N].rearrange(
                "p (t n) -> p t n", t=T)
            x1_sb = x1t_sb
            hTc_sb = act_sb[:, 3 * T * N + T:3 * T * N + T + 2 * N].rearrange(
                "p (s n) -> p s n", s=2)
            cells_sb = c32_sb[:, 0:256]
            gf_sb = c32_sb[:, 256:512]
            bf_sb = c32_sb[:, 512:768]
            wat_sb = c32_sb[:, 768:1024]
            ln_t = {nm: c128_sb[:, C_LN0 + i * T:C_LN0 + (i + 1) * T]
                    for i, nm in enumerate(_LN)}
            ones8_sb = c128_sb[0:8, C_ONE:C_ONE + 1]
            ones32_sb = c128_sb[0:N, C_ONE:C_ONE + 1]
            onesr_sb = c128_sb[0:1, C_ONESR:C_ONESR + 128]
            magic_i = c128_sb[0:N, C_MAGIC:C_MAGIC + 1].bitcast(I32)
            sel32_sb = c128_sb[0:8, C_SEL:C_SEL + 32]

            # ---------------- attention: ai, logits ----------------
            ps_ai = ps.tile([N, S], F32, name="ps_ai", tag="pA")
            for kt in range(KT):
                act = hT_sb if kt < T else eT32_sb
                nc.tensor.matmul(ps_ai[:], act[:, kt % T, :],
                                 wai_sb[:, kt * S:(kt + 1) * S],
                                 start=(kt == 0), stop=(kt == KT - 1))
            ai_sb = sb.tile([N, S], F32, name="ai_sb")
            nc.scalar.activation(ai_sb[:], ps_ai[:], AF.Tanh)
            aw_sb = sb.tile([N, S], F32, name="aw_sb")
            # logits staged 128-partition-wide (zero-padded) so the DRAM
            # staging DMA completes in ~1us instead of ~9us
            lg_pad = sb.tile([128, 1], F32, name="lg_pad")
            nc.vector.memset(lg_pad[:], 0.0)
            nc.vector.tensor_tensor(aw_sb[:], ai_sb[:], wat_sb[:],
                                    op=ALU.mult)
            nc.vector.tensor_reduce(lg_pad[0:N, :], aw_sb[:],
                                    mybir.AxisListType.X, ALU.add)

            # ---------------- AG1: partial logits ----------------
            ag1_in = dram.tile([1, 128], F32, name="ag1_in")
            ag1_out = dram.tile([8, 128], F32, name="ag1_out")
            nc.scalar.dma_start(
                ag1_in[0, :].rearrange("(p one) -> p one", one=1),
                lg_pad[:])
            nc.gpsimd.collective_compute(
                "AllGather", ALU.bypass,
                replica_groups=[list(range(NC))],
                ins=[ag1_in.opt()], outs=[ag1_out.opt()])

            # speculative per-child merge projections (no attention needed):
            # M[p, t, n] = sum_in W_merge[t*128+p, in] * h[n, in], in-chunk
            ps_M = ps.tile([128, T, N], F32, name="ps_M", tag="pD")
            for t in range(T):
                for s in range(2):
                    nc.tensor.matmul(
                        ps_M[:, t, :],
                        wmg_sb[:, s * H + t * 128: s * H + (t + 1) * 128],
                        hTc_sb[:, s, :],
                        start=(s == 0), stop=(s == 1))
            M_sb = sb.tile([128, T, N], F32, name="M_sb")
            nc.vector.tensor_copy(M_sb[:], ps_M[:])

            # ---------------- f_lin + per-child stats ----------------
            ps_f = ps.tile([N, S], F32, name="ps_f", tag="pG")
            for kt in range(KT):
                act = hT_sb if kt < T else xT32_sb
                nc.tensor.matmul(ps_f[:], act[:, kt % T, :],
                                 wf_sb[:, kt * S:(kt + 1) * S],
                                 start=(kt == 0), stop=(kt == KT - 1))
            f_lin_sb = sb.tile([N, S], F32, name="f_lin_sb")
            fstats_sb = sb.tile([N, 2], F32, name="fstats_sb")
            fsq_scr = sb.tile([N, S], F32, name="fsq_scr")
            nc.vector.tensor_copy(f_lin_sb[:], ps_f[:])
            nc.vector.tensor_reduce(fstats_sb[:, 0:1], f_lin_sb[:],
                                    mybir.AxisListType.X, ALU.add)
            nc.vector.scalar_tensor_tensor(fsq_scr[:], f_lin_sb[:], 1.0,
                                           f_lin_sb[:], op0=ALU.mult,
                                           op1=ALU.mult,
                                           accum_out=fstats_sb[:, 1:2])

            # ---------------- iou x-half (off critical path) ------------
            # two single-bank PSUM tiles, one accumulation group each
            ps_iou_a = ps.tile([1, 512], F32, name="ps_iou_a", tag="pIA")
            ps_iou_b = ps.tile([1, 256], F32, name="ps_iou_b", tag="pIB")

            def iou_mm(w_sb, kt, lhs, start, stop):
                nc.tensor.matmul(ps_iou_a[:],
                                 lhs, w_sb[:, kt * 768:kt * 768 + 512],
                                 start=start, stop=stop)
                nc.tensor.matmul(ps_iou_b[:],
                                 lhs, w_sb[:, kt * 768 + 512:
                                           kt * 768 + 768],
                                 start=start, stop=stop)

            for kt in range(T):
                iou_mm(wix_sb, kt, x1_sb[:, kt:kt + 1], kt == 0, False)


            # ---------------- AR2 staging: f stats first ----------------
            ar2_in = dram.tile([1, H + 2 * N], F32, name="ar2_in")
            ar2_out = dram.tile([1, H + 2 * N], F32, name="ar2_out")
            nc.scalar.dma_start(
                ar2_in[0, H:H + 2 * N].rearrange("(k n) -> n k", n=N),
                fstats_sb[:])

            # ---------------- post-AG1: exps, merge partials -------------
            ag1_sb = sb.tile([8, N], F32, name="ag1_sb")
            nc.sync.dma_start(ag1_sb[:], ag1_out[:, 0:N])
            ps_l2r = ps.tile([1, N], F32, name="ps_l2r", tag="pB")
            nc.tensor.matmul(ps_l2r[:], ones8_sb, ag1_sb[:],
                             start=True, stop=True)
            # softmax exp without max-subtraction or normalization (scale
            # cancels in the merge LayerNorm); exp(x) = (1+t)/(1-t) with
            # t = tanh(x/2) keeps everything on the resident tanh table
            th_row = sb.tile([1, N], F32, name="th_row")
            nc.scalar.activation(th_row[:], ps_l2r[:], AF.Tanh, scale=0.5)
            one_m = sb.tile([1, N], F32, name="one_m")
            one_p = sb.tile([1, N], F32, name="one_p")
            exps_row = sb.tile([1, N], F32, name="exps_row")
            nc.vector.tensor_scalar(one_m[:], th_row[:], -1.0, 1.0,
                                    op0=ALU.mult, op1=ALU.add)
            nc.vector.tensor_scalar_add(one_p[:], th_row[:], 1.0)
            nc.vector.reciprocal(one_m[:], one_m[:])
            nc.vector.tensor_tensor(exps_row[:], one_p[:], one_m[:],
                                    op=ALU.mult)
            ps_eb = ps.tile([128, N], F32, name="ps_eb", tag="pH")
            nc.tensor.matmul(ps_eb[:], onesr_sb, exps_row[:],
                             start=True, stop=True)
            exps_b = sb.tile([128, N], F32, name="exps_b")
            nc.vector.tensor_copy(exps_b[:], ps_eb[:])

            # merge-linear partials: one broadcast multiply + one reduce
            mp_sb = sb.tile([128, T], F32, name="mp_sb")
            mp_scr3 = sb.tile([128, T, N], F32, name="mp_scr3")
            eb3 = exps_b[:].rearrange("p (one n) -> p one n",
                                      one=1).to_broadcast((128, T, N))
            nc.vector.tensor_tensor(mp_scr3[:], M_sb[:], eb3, op=ALU.mult)
            nc.vector.tensor_reduce(mp_sb[:], mp_scr3[:],
                                    mybir.AxisListType.X, ALU.add)

            # ---------------- AR2: merge partials + f stats --------------
            mp_dma = nc.scalar.dma_start(
                ar2_in[0, 0:H].rearrange("(p t) -> p t", p=128), mp_sb[:])
            nc.gpsimd.collective_compute(
                "AllReduce", ALU.add,
                replica_groups=[list(range(NC))],
                ins=[ar2_in.opt()], outs=[ar2_out.opt()])

            # ---------------- post-AR2: merge LN + f gate ----------------
            ml_sb = sb.tile([128, T], F32, name="ml_sb")
            fst_t = sb.tile([N, 2], F32, name="fst_t")
            ml_dma = nc.sync.dma_start(
                ml_sb[:], ar2_out[0, 0:H].rearrange("(p t) -> p t", p=128))
            nc.sync.dma_start(
                fst_t[:],
                ar2_out[0, H:H + 2 * N].rearrange("(k n) -> n k", n=N))

            # PE warming during the AR2 mesh: keeps the HAM clock at 8/8 so
            # the iou mh-half matmuls right after AR2 stream at 2.4GHz.
            warm_mms = []
            if WARM_MMS:
                ps_wrm = ps.tile([128, 512], F32, name="ps_wrm", tag="pA")
                NWARM = 14
                for k in range(NWARM):
                    m = nc.tensor.matmul(ps_wrm[:], wim_sb[:, 0:128],
                                         wim_sb[:, k * 512:(k + 1) * 512],
                                         start=(k == 0), stop=(k == NWARM - 1))
                    warm_mms.append(m)
                add_dep_helper(warm_mms[0].ins, ml_dma.ins, sync=True,
                               reason="PE warmup right before iou mh half")
                wsink = sb.tile([1, 1], F32, name="wsink")
                nc.vector.tensor_copy(wsink[:], ps_wrm[0:1, 0:1])
                nc.sync.dma_start(warm_sink[:], wsink[:])


            # merge LayerNorm: one fused GpSimd instruction, then tanh
            mh_pre = sb.tile([128, T], F32, name="mh_pre")
            nc.gpsimd.layernorm(mh_pre[:], ml_sb[:],
                                gamma_ap=ln_t["gm"], beta_ap=ln_t["bm"],
                                eps=EPS, subtract_mean=True)
            mh_bf = sb.tile([128, T], BF16, name="mh_bf")
            nc.scalar.activation(mh_bf[:], mh_pre[:], AF.Tanh)

            # f per-child LayerNorm stats -> rsqrt via bit-trick + Newton
            fmean = sb.tile([N, 1], F32, name="fmean")
            fvar = sb.tile([N, 1], F32, name="fvar")
            fscr = sb.tile([N, 1], F32, name="fscr")
            frstd = sb.tile([N, 1], F32, name="frstd")
            ji = sb.tile([N, 1], I32, name="ji")
            y0b = sb.tile([N, 1], I32, name="y0b")
            nc.vector.tensor_scalar_mul(fmean[:], fst_t[:, 0:1], INV_H)
            nc.vector.tensor_scalar_mul(fvar[:], fst_t[:, 1:2], INV_H)
            nc.vector.tensor_tensor(fscr[:], fmean[:], fmean[:],
                                    op=ALU.mult)
            nc.vector.tensor_sub(fvar[:], fvar[:], fscr[:])
            nc.vector.tensor_scalar_add(fvar[:], fvar[:], EPS)
            nc.vector.tensor_scalar(ji[:], fvar[:].bitcast(I32), 1, None,
                                    op0=ALU.arith_shift_right)
            nc.vector.tensor_tensor(y0b[:], magic_i, ji[:], op=ALU.subtract)
            y = y0b[:].bitcast(F32)
            for _ in range(2):  # Newton: y *= 1.5 - 0.5*v*y*y
                nc.vector.tensor_tensor(fscr[:], fvar[:], y, op=ALU.mult)
                nc.vector.tensor_tensor(fscr[:], fscr[:], y, op=ALU.mult)
                nc.vector.tensor_scalar(fscr[:], fscr[:], -0.5, 1.5,
                                        op0=ALU.mult, op1=ALU.add)
                nc.vector.tensor_tensor(y, y, fscr[:], op=ALU.mult)
            nc.vector.tensor_copy(frstd[:], y)

            ft = sb.tile([N, S], F32, name="ft")
            nc.vector.tensor_scalar(ft[:], f_lin_sb[:], fmean[:], frstd[:],
                                    op0=ALU.subtract, op1=ALU.mult)
            nc.vector.tensor_tensor(ft[:], ft[:], gf_sb, op=ALU.mult)
            nc.vector.tensor_tensor(ft[:], ft[:], bf_sb, op=ALU.add)
            f_sb = sb.tile([N, S], F32, name="f_sb")
            nc.scalar.activation(f_sb[:], ft[:], AF.Sigmoid)
            fprod = sb.tile([N, S], F32, name="fprod")
            nc.vector.tensor_tensor(fprod[:], f_sb[:], cells_sb,
                                    op=ALU.mult)
            ps_fc = ps.tile([128, 2], F32, name="ps_fc", tag="pB")
            for s in range(2):
                nc.tensor.matmul(ps_fc[:, s:s + 1],
                                 fprod[:, s * 128:(s + 1) * 128],
                                 ones32_sb, start=True, stop=True)
            fc_sb = sb.tile([128, 2], F32, name="fc_sb")
            nc.vector.tensor_copy(fc_sb[:], ps_fc[:])

            # ---------------- AG3 staging: fc chunk first ----------------
            ag3_in = dram.tile([1, 4 * S], F32, name="ag3_in")
            ag3_out = dram.tile([8, 4 * S], F32, name="ag3_out")
            nc.scalar.dma_start(
                ag3_in[0, 3 * S:4 * S].rearrange("(s p) -> p s", s=2),
                fc_sb[:])

            # ---------------- iou mh-half (post-AR2 PE work) -------------
            for kt in range(T, KT):
                iou_mm(wim_sb, kt - T, mh_bf[:, kt - T:kt - T + 1],
                       False, kt == KT - 1)
            iou_sb = sb.tile([1, 3 * S], F32, name="iou_sb")
            nc.vector.tensor_copy(iou_sb[:, 0:512], ps_iou_a[:])
            nc.vector.tensor_copy(iou_sb[:, 512:768], ps_iou_b[:])
            # transpose [1,768] -> [128,6] with K=1 matmuls so the staging
            # DMA runs 128-partition-wide (single-partition DMAs to DRAM
            # have ~9us completion latency)
            ps_iout = ps.tile([128, 6], F32, name="ps_iout", tag="pB")
            for j in range(6):
                nc.tensor.matmul(ps_iout[:, j:j + 1],
                                 iou_sb[0:1, j * 128:(j + 1) * 128],
                                 c128_sb[0:1, C_ONE:C_ONE + 1],
                                 start=(j == 0), stop=(j == 5))
            iout_sb = sb.tile([128, 6], F32, name="iout_sb")
            nc.vector.tensor_copy(iout_sb[:], ps_iout[:])
            nc.scalar.dma_start(
                ag3_in[0, 0:3 * S].rearrange("(j p) -> p j", p=128),
                iout_sb[:])

            # ---------------- AG3: iou chunk + fc chunk ----------------
            nc.gpsimd.collective_compute(
                "AllGather", ALU.bypass,
                replica_groups=[list(range(NC))],
                ins=[ag3_in.opt()], outs=[ag3_out.opt()])

            # contiguous readback + one-hot selector matmuls into t-major
            ag3_sb = sb.tile([8, 4 * S], F32, name="ag3_sb")
            nc.sync.dma_start(ag3_sb[:], ag3_out[:])
            vec_sb = []
            for v in range(4):
                pv = ps.tile([128, T], F32, name=f"ps_vec{v}",
                             tag=["pIA", "pIB", "pD", "pH"][v])
                for half in range(2):
                    k = v * 2 + half
                    nc.tensor.matmul(
                        pv[:], ag3_sb[:, k * 128:(k + 1) * 128],
                        sel32_sb[:, half * T:(half + 1) * T],
                        start=(half == 0), stop=(half == 1))
                vs = sb.tile([128, T], F32, name=f"vec{v}")
                nc.vector.tensor_copy(vs[:], pv[:])
                vec_sb.append(vs)

            def flat(ap):
                return ap[:]

            # ---------------- i/o/u gates ----------------
            gates = []
            for v, (g_nm, b_nm, fn, nm) in enumerate([
                    ("gi", "bi", AF.Sigmoid, "ig"),
                    ("go", "bo", AF.Sigmoid, "og"),
                    ("gu", "bu", AF.Tanh, "ug")]):
                lnv = sb.tile([128, T], F32, name=nm + "_ln")
                nc.gpsimd.layernorm(lnv[:], vec_sb[v][:],
                                    gamma_ap=ln_t[g_nm], beta_ap=ln_t[b_nm],
                                    eps=EPS, subtract_mean=True)
                out = sb.tile([128, T], F32, name=nm)
                nc.scalar.activation(out[:], lnv[:], fn)
                gates.append(out)
            i_sb, o_sb, u_sb = gates

            cell_lin = sb.tile([128, T], F32, name="cell_lin")
            nc.vector.tensor_tensor(cell_lin[:], i_sb[:], u_sb[:],
                                    op=ALU.mult)
            nc.vector.tensor_tensor(cell_lin[:], cell_lin[:],
                                    vec_sb[3][:], op=ALU.add)
            new_c = sb.tile([128, T], F32, name="new_c")
            nc.gpsimd.layernorm(new_c[:], cell_lin[:],
                                gamma_ap=ln_t["gc"], beta_ap=ln_t["bc"],
                                eps=EPS, subtract_mean=True)
            th = sb.tile([128, T], F32, name="th")
            nc.scalar.activation(th[:], new_c[:], AF.Tanh)
            new_h = sb.tile([128, T], F32, name="new_h")
            nc.vector.tensor_tensor(new_h[:], o_sb[:], th[:], op=ALU.mult)

            nc.sync.dma_start(out_c[:], new_c[:])
            nc.scalar.dma_start(out_h[:], new_h[:])

            if dbg:
                for nm, src in [("d_lg", lg_pad[0:N, :]), ("d_exps", exps_row),
                                ("d_ml", ml_sb), ("d_mh", mh_pre),
                                ("d_frstd", frstd), ("d_f", f_sb),
                                ("d_fc", fc_sb), ("d_iou", iou_sb),
                                ("d_lni", gates[0]), ("d_cl", cell_lin)]:
                    dd = sb.tile(list(dbg_t[nm].shape), F32, name=nm + "_d")
                    nc.vector.tensor_copy(dd[:], src[:])
                    nc.sync.dma_start(dbg_t[nm][:], dd[:])
                dmh2 = sb.tile([128, T], F32, name="d_mhbf_d")
                nc.vector.tensor_copy(dmh2[:], mh_bf[:])
                nc.sync.dma_start(dbg_t["d_mhbf"][:], dmh2[:])
                for nm, src in [("d_vec0", vec_sb[0]), ("d_vec3", vec_sb[3])]:
                    dd = sb.tile([128, 16], F32, name=nm + "_d")
                    nc.vector.tensor_copy(dd[:], src[:])
                    nc.sync.dma_start(dbg_t[nm][:], dd[:])

    nc.compile()
    return nc


def _tmaj(v):
    """[2048] vector -> [128,16] t-major sbuf image (sb[p,t] = v[t*128+p])."""
    return np.ascontiguousarray(v.reshape(T, 128).T)


def _scmaj(v):
    """[2048] vector -> [128,16] sc-major image (sb[p,s*8+c] = v[c*256+s*128+p])."""
    return np.ascontiguousarray(
        v.reshape(8, 2, 128).transpose(2, 1, 0).reshape(128, 16))


def _ktiles(wT, cols):
    """wT: [K_in, out_cols] -> [128, (K_in/128)*cols] partition-major pack."""
    k_in = wT.shape[0]
    return np.ascontiguousarray(
        wT.reshape(k_in // 128, 128, cols).transpose(1, 0, 2).reshape(
            128, (k_in // 128) * cols))


def kernel(input, hiddens, cells, external,
           W_ai, W_attn, W_merge, W_iou, W_fi, W_fh,
           g_merge, b_merge, g_f, b_f, g_i, b_i, g_o, b_o, g_u, b_u,
           g_c, b_c, _dbg=False):
    key = ("nc", _dbg)
    if key not in _CACHE:
        _CACHE[key] = _build(_dbg)
    nc = _CACHE[key]

    f32 = np.float32
    input = np.asarray(input, f32)
    hiddens = np.asarray(hiddens, f32)
    cells = np.asarray(cells, f32)
    external = np.asarray(external, f32)

    hTt = _ktiles(np.ascontiguousarray(hiddens.T), N)
    xT32 = _ktiles(np.tile(input[:, None], (1, N)), N)
    eT32 = _ktiles(np.tile(external[:, None], (1, N)), N)
    x1 = _tmaj(input)

    cpack128 = np.zeros((128, CP128), f32)
    for i, v in enumerate([g_merge, b_merge, g_i, b_i, g_o, b_o,
                           g_u, b_u, g_c, b_c]):
        cpack128[:, C_LN0 + i * T:C_LN0 + (i + 1) * T] = _tmaj(
            np.asarray(v, f32))
    cpack128[:, C_ONE] = 1.0
    cpack128[0, C_ONESR:C_ONESR + 128] = 1.0
    cpack128[:, C_MAGIC] = np.full(128, MAGIC, np.int32).view(f32)
    for c in range(8):
        for h2 in range(2):
            cpack128[c, C_SEL + h2 * T + 2 * c + h2] = 1.0

    Wf_cat = np.concatenate([W_fh, W_fi], axis=1)              # [H, 4096]
    in_maps = []
    for c in range(NC):
        r = slice(c * S, (c + 1) * S)
        iou_rows = np.concatenate(
            [W_iou[g * H + c * S:g * H + (c + 1) * S, :] for g in range(3)],
            axis=0)                                            # [768, 4096]
        wiou_full = _ktiles(np.ascontiguousarray(iou_rows.T), 3 * S)
        actpack = np.concatenate([
            hTt, xT32, eT32, x1,
            np.ascontiguousarray(
                hiddens.T[c * S:(c + 1) * S].reshape(2, 128, N)
                .transpose(1, 0, 2).reshape(128, 2 * N)),
        ], axis=1).astype(NPBF)
        cpack32 = np.concatenate([
            np.ascontiguousarray(cells[:, r]),
            np.tile(np.asarray(g_f, f32)[r], (N, 1)),
            np.tile(np.asarray(b_f, f32)[r], (N, 1)),
            np.tile(np.asarray(W_attn, f32)[0, r], (N, 1)),
        ], axis=1).astype(f32)
        m = {
            "actpack": np.ascontiguousarray(actpack),
            "x1d": np.ascontiguousarray(x1.astype(NPBF)),
            "cpack32": np.ascontiguousarray(cpack32),
            "cpack128": cpack128,
            "wai": _ktiles(np.ascontiguousarray(W_ai[r].T), S).astype(NPBF),
            "wf": _ktiles(np.ascontiguousarray(Wf_cat[r].T), S).astype(NPBF),
            "wmg": _ktiles(np.ascontiguousarray(W_merge[:, r].T),
                           H).astype(NPBF),
            "wioux": np.ascontiguousarray(
                wiou_full[:, :T * 3 * S]).astype(NPBF),
            "wioumh": np.ascontiguousarray(
                wiou_full[:, T * 3 * S:]).astype(NPBF),
        }
        in_maps.append(m)

    res = run_bass_kernel_spmd(nc, in_maps, core_ids=list(range(NC)))
    _CACHE["last_results"] = res
    r0 = res.results[0]
    new_h = np.ascontiguousarray(r0["out_h"].T).reshape(H)
    new_c = np.ascontiguousarray(r0["out_c"].T).reshape(H)
    return new_h, new_c


# revision 23
# speedup vs baseline: 1.1331x; 1.1331x over previous
"""AttentiveChildSumTreeLSTMCell on 8 Trainium2 NeuronCores.

Tensor-parallel: column-parallel f/attention/iou linears (hidden dim sharded
8 ways), row-parallel merge linear.  Cross-core exchanges: a warmup collective
triggered immediately at kernel start (absorbs the ~60us first-collective
init barrier), AllGather of partial attention logits, AllReduce of
merge-linear partials + f LayerNorm stats, AllGather of iou/forget*cell
chunks.

Perf notes vs the naive schedule:
- All inputs are packed host-side into 9 large DMAs (one bf16 activation
  pack, two fp32 constant packs, five weight streams) to kill the ~0.65us
  per-DMA dispatch serialization on the Sync queue.
- Every activation uses the sigmoid_and_others ACT table set (sigmoid+tanh):
  softmax exp is computed as (1+t)/(1-t) with t=tanh(x/2); LayerNorm rsqrt
  for the f gate uses a Quake-style bit-trick seed + 2 Newton steps on DVE.
  One table load for the whole kernel, zero mid-kernel ACT_TABLE_LOADs.
- The four full-width (2048) LayerNorms (merge, i, o, u, cell) run as single
  fused InstLayernorm instructions on GpSimd.
- AG3 results are read back with rearranged-AP DMAs directly into t-major
  [128,16] tiles (no one-hot selector matmuls).
- Dummy matmuls during the AR2 window keep the PE HAM clock warm so the
  post-AR2 iou matmuls stream at 2.4GHz instead of 1.2GHz.
"""

import sys

for _p in ("/opt/trn_rl_repo",):
    if _p not in sys.path:
        sys.path.insert(0, _p)

import ml_dtypes
import numpy as np

import concourse.bacc as bacc
import concourse.mybir as mybir
import concourse.tile as tile
from concourse.bass_utils import run_bass_kernel_spmd
from concourse.tile_rust import add_dep_helper

F32 = mybir.dt.float32
BF16 = mybir.dt.bfloat16
I32 = mybir.dt.int32
AF = mybir.ActivationFunctionType
ALU = mybir.AluOpType
NPBF = ml_dtypes.bfloat16

H = 2048
N = 32
NC = 8
S = H // NC           # 256: per-core chunk of every sharded dim
T = H // 128          # 16 tiles of 128 along a 2048 dim
KT = 32               # K-tiles along the 4096 contraction dims
EPS = 1e-5
INV_H = 1.0 / H
MAGIC = 0x5F3759DF    # rsqrt bit-trick seed constant
WARM_MMS = True      # PE HAM warm-up dummies during the AR2 window

# cpack128 column layout (fp32 [128, CP128])
_LN = ["gm", "bm", "gi", "bi", "go", "bo", "gu", "bu", "gc", "bc"]
C_LN0 = 0             # 10 * T cols of LayerNorm gamma/beta
C_ONE = C_LN0 + 10 * T    # 160: ones column
C_ONESR = C_ONE + 1       # 161..289: row 0 = ones[128] (bcast row)
C_MAGIC = C_ONESR + 128   # 289: rsqrt magic bits
C_SEL = C_MAGIC + 1       # 290..322: sel32 one-hot reassembly (rows 0-7)
CP128 = C_SEL + 32        # 322

_CACHE = {}


def _build(dbg=False):
    nc = bacc.Bacc(None, target_bir_lowering=False, debug=False,
                   num_devices=NC)

    def din(name, shape, dt=F32):
        return nc.dram_tensor(name, list(shape), dt, kind="ExternalInput")

    # ---- per-core DRAM inputs (SPMD: same shapes on every core) ----
    actpack = din("actpack", (128, 1616), BF16)   # hT|xT32|eT32|x1|hTc
    cpack32 = din("cpack32", (32, 1024))          # cells|gf|bf|wattn
    cpack128 = din("cpack128", (128, CP128))      # LN g/b, ones, onesr, magic
    wai = din("wai", (128, KT * S), BF16)         # W_ai^T chunk
    wf = din("wf", (128, KT * S), BF16)           # [W_fh | W_fi]^T chunk
    wmg = din("wmg", (128, 2 * H), BF16)          # W_merge^T in-chunk
    x1d = din("x1d", (128, T), BF16)              # x t-major
    wioux = din("wioux", (128, T * 3 * S), BF16)  # W_iou^T x-half
    wioumh = din("wioumh", (128, T * 3 * S), BF16)  # W_iou^T mh-half

    out_h = nc.dram_tensor("out_h", [128, T], F32, kind="ExternalOutput")
    out_c = nc.dram_tensor("out_c", [128, T], F32, kind="ExternalOutput")
    warm_sink = nc.dram_tensor("warm_sink", [1, 1], F32,
                               kind="ExternalOutput")
    dbg_t = {}
    if dbg:
        for nm, shp in [("d_lg", [N, 1]), ("d_exps", [1, N]),
                        ("d_ml", [128, T]), ("d_mh", [128, T]),
                        ("d_frstd", [N, 1]), ("d_f", [N, S]),
                        ("d_fc", [128, 2]), ("d_iou", [1, 3 * S]),
                        ("d_vec0", [128, 16]), ("d_vec3", [128, 16]),
                        ("d_ioux", [1, 3 * S]), ("d_mhbf", [128, T]),
                        ("d_lni", [128, T]), ("d_cl", [128, T])]:
            dbg_t[nm] = nc.dram_tensor(nm, shp, F32, kind="ExternalOutput")

    with tile.TileContext(nc) as tc:
        with (
            tc.tile_pool(name="sb", bufs=1) as sb,
            tc.tile_pool(name="ps", bufs=1, space="PSUM") as ps,
            tc.tile_pool(name="dram", bufs=1, space="DRAM") as dram,
        ):
            # ------- warmup collective: fires instantly on garbage DRAM ----
            # (content is irrelevant; its only job is to pay the one-time
            # communicator/channel init + cross-core start alignment while
            # the weight DMAs stream)
            warm_in = dram.tile([1, 64], F32, name="warm_in")
            warm_out = dram.tile([8, 64], F32, name="warm_out")
            nc.gpsimd.collective_compute(
                "AllGather", ALU.bypass,
                replica_groups=[list(range(NC))],
                ins=[warm_in.opt()], outs=[warm_out.opt()])

            # ------- ACT table preload: one dummy sigmoid ----
            # all activations in this kernel are sigmoid/tanh -> one set
            tl_scr = sb.tile([1, 1], F32, name="tl_scr")
            nc.vector.memset(tl_scr[:], 0.5)
            nc.scalar.activation(tl_scr[:], tl_scr[:], AF.Sigmoid)

            # ---------------- input DMAs (ordered, 2-deep chained) --------
            act_sb = sb.tile([128, 1616], BF16, name="act_sb")
            x1t_sb = sb.tile([128, T], BF16, name="x1t_sb")
            c32_sb = sb.tile([32, 1024], F32, name="c32_sb")
            c128_sb = sb.tile([128, CP128], F32, name="c128_sb")
            wai_sb = sb.tile([128, KT * S], BF16, name="wai_sb")
            wf_sb = sb.tile([128, KT * S], BF16, name="wf_sb")
            wmg_sb = sb.tile([128, 2 * H], BF16, name="wmg_sb")
            wix_sb = sb.tile([128, T * 3 * S], BF16, name="wix_sb")
            wim_sb = sb.tile([128, T * 3 * S], BF16, name="wim_sb")

            nc.sync.dma_start(x1t_sb[:], x1d[:])
            wdmas = []
            for dst, src in ((act_sb, actpack), (wai_sb, wai),
                             (c32_sb, cpack32), (c128_sb, cpack128),
                             (wf_sb, wf), (wmg_sb, wmg),
                             (wix_sb, wioux), (wim_sb, wioumh)):
                wdmas.append(nc.sync.dma_start(dst[:], src[:]))
            for i in range(2, len(wdmas)):
                add_dep_helper(wdmas[i].ins, wdmas[i - 2].ins, sync=True,
                               reason="weight DMA arrival order")

            # subviews of the packs
            hT_sb = act_sb[:].rearrange(
                "p (x) -> p x")[:, 0:T * N].rearrange(
                "p (t n) -> p t n", t=T)
            xT32_sb = act_sb[:, T * N:2 * T * N].rearrange(
                "p (t n) -> p t n", t=T)
            eT32_sb = act_sb[:, 2 * T * N:3 * T * N].rearrange(
                "p (t n) -> p t n", t=T)
            x1_sb = x1t_sb
            hTc_sb = act_sb[:, 3 * T * N + T:3 * T * N + T + 2 * N].rearrange(
                "p (s n) -> p s n", s=2)
            cells_sb = c32_sb[:, 0:256]
            gf_sb = c32_sb[:, 256:512]
            bf_sb = c32_sb[:, 512:768]
            wat_sb = c32_sb[:, 768:1024]
            ln_t = {nm: c128_sb[:, C_LN0 + i * T:C_LN0 + (i + 1) * T]
                    for i, nm in enumerate(_LN)}
            ones8_sb = c128_sb[0:8, C_ONE:C_ONE + 1]
            ones32_sb = c128_sb[0:N, C_ONE:C_ONE + 1]
            onesr_sb = c128_sb[0:1, C_ONESR:C_ONESR + 128]
            magic_i = c128_sb[0:N, C_MAGIC:C_MAGIC + 1].bitcast(I32)
            sel32_sb = c128_sb[0:8, C_SEL:C_SEL + 32]

            # ---------------- attention: ai, logits ----------------
            ps_ai = ps.tile([N, S], F32, name="ps_ai", tag="pA")
            for kt in range(KT):
                act = hT_sb if kt < T else eT32_sb
                nc.tensor.matmul(ps_ai[:], act[:, kt % T, :],
                                 wai_sb[:, kt * S:(kt + 1) * S],
                                 start=(kt == 0), stop=(kt == KT - 1))
            ai_sb = sb.tile([N, S], F32, name="ai_sb")
            nc.scalar.activation(ai_sb[:], ps_ai[:], AF.Tanh)
            aw_sb = sb.tile([N, S], F32, name="aw_sb")
            # logits staged 128-partition-wide (zero-padded) so the DRAM
            # staging DMA completes in ~1us instead of ~9us
            lg_pad = sb.tile([128, 8], F32, name="lg_pad")
            nc.vector.memset(lg_pad[:], 0.0)
            nc.vector.tensor_tensor(aw_sb[:], ai_sb[:], wat_sb[:],
                                    op=ALU.mult)
            nc.vector.tensor_reduce(lg_pad[0:N, 0:1], aw_sb[:],
                                    mybir.AxisListType.X, ALU.add)

            # ---------------- AG1: partial logits ----------------
            ag1_in = dram.tile([1, 1024], F32, name="ag1_in")
            ag1_out = dram.tile([8, 1024], F32, name="ag1_out")
            nc.scalar.dma_start(
                ag1_in[0, :].rearrange("(p j) -> p j", j=8),
                lg_pad[:])
            nc.gpsimd.collective_compute(
                "AllGather", ALU.bypass,
                replica_groups=[list(range(NC))],
                ins=[ag1_in.opt()], outs=[ag1_out.opt()])

            # speculative per-child merge projections (no attention needed):
            # M[p, t, n] = sum_in W_merge[t*128+p, in] * h[n, in], in-chunk
            ps_M = ps.tile([128, T, N], F32, name="ps_M", tag="pD")
            for t in range(T):
                for s in range(2):
                    nc.tensor.matmul(
                        ps_M[:, t, :],
                        wmg_sb[:, s * H + t * 128: s * H + (t + 1) * 128],
                        hTc_sb[:, s, :],
                        start=(s == 0), stop=(s == 1))
            M_sb = sb.tile([128, T, N], F32, name="M_sb")
            nc.vector.tensor_copy(M_sb[:], ps_M[:])

            # ---------------- f_lin + per-child stats ----------------
            ps_f = ps.tile([N, S], F32, name="ps_f", tag="pG")
            for kt in range(KT):
                act = hT_sb if kt < T else xT32_sb
                nc.tensor.matmul(ps_f[:], act[:, kt % T, :],
                                 wf_sb[:, kt * S:(kt + 1) * S],
                                 start=(kt == 0), stop=(kt == KT - 1))
            f_lin_sb = sb.tile([N, S], F32, name="f_lin_sb")
            fstats_sb = sb.tile([N, 2], F32, name="fstats_sb")
            fsq_scr = sb.tile([N, S], F32, name="fsq_scr")
            nc.vector.tensor_copy(f_lin_sb[:], ps_f[:])
            nc.vector.tensor_reduce(fstats_sb[:, 0:1], f_lin_sb[:],
                                    mybir.AxisListType.X, ALU.add)
            nc.vector.scalar_tensor_tensor(fsq_scr[:], f_lin_sb[:], 1.0,
                                           f_lin_sb[:], op0=ALU.mult,
                                           op1=ALU.mult,
                                           accum_out=fstats_sb[:, 1:2])

            # ---------------- iou x-half (off critical path) ------------
            # two single-bank PSUM tiles, one accumulation group each
            ps_iou_a = ps.tile([1, 512], F32, name="ps_iou_a", tag="pIA")
            ps_iou_b = ps.tile([1, 256], F32, name="ps_iou_b", tag="pIB")

            def iou_mm(w_sb, kt, lhs, start, stop):
                nc.tensor.matmul(ps_iou_a[:],
                                 lhs, w_sb[:, kt * 768:kt * 768 + 512],
                                 start=start, stop=stop)
                nc.tensor.matmul(ps_iou_b[:],
                                 lhs, w_sb[:, kt * 768 + 512:
                                           kt * 768 + 768],
                                 start=start, stop=stop)

            for kt in range(T):
                iou_mm(wix_sb, kt, x1_sb[:, kt:kt + 1], kt == 0, False)


            # ---------------- AR2 staging: f stats first ----------------
            ar2_in = dram.tile([1, H + 2 * N], F32, name="ar2_in")
            ar2_out = dram.tile([1, H + 2 * N], F32, name="ar2_out")
            nc.scalar.dma_start(
                ar2_in[0, H:H + 2 * N].rearrange("(k n) -> n k", n=N),
                fstats_sb[:])

            # ---------------- post-AG1: exps, merge partials -------------
            ag1_sb = sb.tile([8, 1024], F32, name="ag1_sb")
            nc.sync.dma_start(ag1_sb[:], ag1_out[:])
            ag1_lg = ag1_sb[:, 0:N * 8].rearrange("c (n j) -> c n j",
                                                  j=8)[:, :, 0]
            ps_l2r = ps.tile([1, N], F32, name="ps_l2r", tag="pB")
            nc.tensor.matmul(ps_l2r[:], ones8_sb, ag1_lg,
                             start=True, stop=True)
            # softmax exp without max-subtraction or normalization (scale
            # cancels in the merge LayerNorm); exp(x) = (1+t)/(1-t) with
            # t = tanh(x/2) keeps everything on the resident tanh table
            th_row = sb.tile([1, N], F32, name="th_row")
            nc.scalar.activation(th_row[:], ps_l2r[:], AF.Tanh, scale=0.5)
            one_m = sb.tile([1, N], F32, name="one_m")
            one_p = sb.tile([1, N], F32, name="one_p")
            exps_row = sb.tile([1, N], F32, name="exps_row")
            nc.vector.tensor_scalar(one_m[:], th_row[:], -1.0, 1.0,
                                    op0=ALU.mult, op1=ALU.add)
            nc.vector.tensor_scalar_add(one_p[:], th_row[:], 1.0)
            nc.vector.reciprocal(one_m[:], one_m[:])
            nc.vector.tensor_tensor(exps_row[:], one_p[:], one_m[:],
                                    op=ALU.mult)
            ps_eb = ps.tile([128, N], F32, name="ps_eb", tag="pH")
            nc.tensor.matmul(ps_eb[:], onesr_sb, exps_row[:],
                             start=True, stop=True)
            exps_b = sb.tile([128, N], F32, name="exps_b")
            nc.vector.tensor_copy(exps_b[:], ps_eb[:])

            # merge-linear partials: one broadcast multiply + one reduce
            mp_sb = sb.tile([128, T], F32, name="mp_sb")
            mp_scr3 = sb.tile([128, T, N], F32, name="mp_scr3")
            eb3 = exps_b[:].rearrange("p (one n) -> p one n",
                                      one=1).to_broadcast((128, T, N))
            nc.vector.tensor_tensor(mp_scr3[:], M_sb[:], eb3, op=ALU.mult)
            nc.vector.tensor_reduce(mp_sb[:], mp_scr3[:],
                                    mybir.AxisListType.X, ALU.add)

            # ---------------- AR2: merge partials + f stats --------------
            mp_dma = nc.scalar.dma_start(
                ar2_in[0, 0:H].rearrange("(p t) -> p t", p=128), mp_sb[:])
            nc.gpsimd.collective_compute(
                "AllReduce", ALU.add,
                replica_groups=[list(range(NC))],
                ins=[ar2_in.opt()], outs=[ar2_out.opt()])

            # ---------------- post-AR2: merge LN + f gate ----------------
            ml_sb = sb.tile([128, T], F32, name="ml_sb")
            fst_t = sb.tile([N, 2], F32, name="fst_t")
            ml_dma = nc.sync.dma_start(
                ml_sb[:], ar2_out[0, 0:H].rearrange("(p t) -> p t", p=128))
            nc.sync.dma_start(
                fst_t[:],
                ar2_out[0, H:H + 2 * N].rearrange("(k n) -> n k", n=N))

            # PE warming during the AR2 mesh: keeps the HAM clock at 8/8 so
            # the iou mh-half matmuls right after AR2 stream at 2.4GHz.
            warm_mms = []
            if WARM_MMS:
                ps_wrm = ps.tile([128, 512], F32, name="ps_wrm", tag="pA")
                NWARM = 14
                for k in range(NWARM):
                    m = nc.tensor.matmul(ps_wrm[:], wim_sb[:, 0:128],
                                         wim_sb[:, k * 512:(k + 1) * 512],
                                         start=(k == 0), stop=(k == NWARM - 1))
                    warm_mms.append(m)
                add_dep_helper(warm_mms[0].ins, ml_dma.ins, sync=True,
                               reason="PE warmup right before iou mh half")
                wsink = sb.tile([1, 1], F32, name="wsink")
                nc.vector.tensor_copy(wsink[:], ps_wrm[0:1, 0:1])
                nc.sync.dma_start(warm_sink[:], wsink[:])


            # merge LayerNorm: one fused GpSimd instruction, then tanh
            mh_pre = sb.tile([128, T], F32, name="mh_pre")
            nc.gpsimd.layernorm(mh_pre[:], ml_sb[:],
                                gamma_ap=ln_t["gm"], beta_ap=ln_t["bm"],
                                eps=EPS, subtract_mean=True)
            mh_bf = sb.tile([128, T], BF16, name="mh_bf")
            nc.scalar.activation(mh_bf[:], mh_pre[:], AF.Tanh)

            # f per-child LayerNorm stats -> rsqrt via bit-trick + Newton
            fmean = sb.tile([N, 1], F32, name="fmean")
            fvar = sb.tile([N, 1], F32, name="fvar")
            fscr = sb.tile([N, 1], F32, name="fscr")
            frstd = sb.tile([N, 1], F32, name="frstd")
            ji = sb.tile([N, 1], I32, name="ji")
            y0b = sb.tile([N, 1], I32, name="y0b")
            nc.vector.tensor_scalar_mul(fmean[:], fst_t[:, 0:1], INV_H)
            nc.vector.tensor_scalar_mul(fvar[:], fst_t[:, 1:2], INV_H)
            nc.vector.tensor_tensor(fscr[:], fmean[:], fmean[:],
                                    op=ALU.mult)
            nc.vector.tensor_sub(fvar[:], fvar[:], fscr[:])
            nc.vector.tensor_scalar_add(fvar[:], fvar[:], EPS)
            nc.vector.tensor_scalar(ji[:], fvar[:].bitcast(I32), 1, None,
                                    op0=ALU.arith_shift_right)
            nc.vector.tensor_tensor(y0b[:], magic_i, ji[:], op=ALU.subtract)
            y = y0b[:].bitcast(F32)
            for _ in range(2):  # Newton: y *= 1.5 - 0.5*v*y*y
                nc.vector.tensor_tensor(fscr[:], fvar[:], y, op=ALU.mult)
                nc.vector.tensor_tensor(fscr[:], fscr[:], y, op=ALU.mult)
                nc.vector.tensor_scalar(fscr[:], fscr[:], -0.5, 1.5,
                                        op0=ALU.mult, op1=ALU.add)
                nc.vector.tensor_tensor(y, y, fscr[:], op=ALU.mult)
            nc.vector.tensor_copy(frstd[:], y)

            ft = sb.tile([N, S], F32, name="ft")
            nc.vector.tensor_scalar(ft[:], f_lin_sb[:], fmean[:], frstd[:],
                                    op0=ALU.subtract, op1=ALU.mult)
            nc.vector.tensor_tensor(ft[:], ft[:], gf_sb, op=ALU.mult)
            nc.vector.tensor_tensor(ft[:], ft[:], bf_sb, op=ALU.add)
            f_sb = sb.tile([N, S], F32, name="f_sb")
            nc.scalar.activation(f_sb[:], ft[:], AF.Sigmoid)
            fprod = sb.tile([N, S], F32, name="fprod")
            nc.vector.tensor_tensor(fprod[:], f_sb[:], cells_sb,
                                    op=ALU.mult)
            ps_fc = ps.tile([128, 2], F32, name="ps_fc", tag="pB")
            for s in range(2):
                nc.tensor.matmul(ps_fc[:, s:s + 1],
                                 fprod[:, s * 128:(s + 1) * 128],
                                 ones32_sb, start=True, stop=True)
            fc_sb = sb.tile([128, 2], F32, name="fc_sb")
            nc.vector.tensor_copy(fc_sb[:], ps_fc[:])

            # ---------------- AG3 staging: fc chunk first ----------------
            # payload layout: [0:1024] iou at flat p*8 + j (j = 2*gate+s),
            # [1024:2048] fc at flat p*8 + s.  8 floats = 32B per partition
            # keeps the staging DMA writes at HBM line rate (no RMW).
            ag3_in = dram.tile([1, 2048], F32, name="ag3_in")
            ag3_out = dram.tile([8, 2048], F32, name="ag3_out")
            fc_pad = sb.tile([128, 8], F32, name="fc_pad")
            nc.vector.tensor_copy(fc_pad[:, 0:2], fc_sb[:])
            nc.scalar.dma_start(
                ag3_in[0, 1024:2048].rearrange("(p j) -> p j", j=8),
                fc_pad[:])

            # ---------------- iou mh-half (post-AR2 PE work) -------------
            for kt in range(T, KT):
                iou_mm(wim_sb, kt - T, mh_bf[:, kt - T:kt - T + 1],
                       False, kt == KT - 1)
            iou_sb = sb.tile([1, 3 * S], F32, name="iou_sb")
            nc.vector.tensor_copy(iou_sb[:, 0:512], ps_iou_a[:])
            nc.vector.tensor_copy(iou_sb[:, 512:768], ps_iou_b[:])
            # transpose [1,768] -> [128,6] with K=1 matmuls so the staging
            # DMA runs 128-partition-wide (single-partition DMAs to DRAM
            # have ~9us completion latency)
            ps_iout = ps.tile([128, 6], F32, name="ps_iout", tag="pB")
            for j in range(6):
                nc.tensor.matmul(ps_iout[:, j:j + 1],
                                 iou_sb[0:1, j * 128:(j + 1) * 128],
                                 c128_sb[0:1, C_ONE:C_ONE + 1],
                                 start=(j == 0), stop=(j == 5))
            iout_sb = sb.tile([128, 8], F32, name="iout_sb")
            nc.vector.tensor_copy(iout_sb[:, 0:6], ps_iout[:])
            nc.scalar.dma_start(
                ag3_in[0, 0:1024].rearrange("(p j) -> p j", j=8),
                iout_sb[:])

            # ---------------- AG3: iou chunk + fc chunk ----------------
            nc.gpsimd.collective_compute(
                "AllGather", ALU.bypass,
                replica_groups=[list(range(NC))],
                ins=[ag3_in.opt()], outs=[ag3_out.opt()])

            # contiguous readback + one-hot selector matmuls into t-major
            ag3_sb = sb.tile([8, 2048], F32, name="ag3_sb")
            nc.sync.dma_start(ag3_sb[:], ag3_out[:])
            ag3_iou = ag3_sb[:, 0:1024].rearrange("c (p j) -> c j p", j=8)
            ag3_fc = ag3_sb[:, 1024:2048].rearrange("c (p j) -> c j p", j=8)
            vec_sb = []
            for v in range(4):
                pv = ps.tile([128, T], F32, name=f"ps_vec{v}",
                             tag=["pIA", "pIB", "pD", "pH"][v])
                for half in range(2):
                    lhs = (ag3_iou[:, v * 2 + half, :] if v < 3
                           else ag3_fc[:, half, :])
                    nc.tensor.matmul(
                        pv[:], lhs,
                        sel32_sb[:, half * T:(half + 1) * T],
                        start=(half == 0), stop=(half == 1))
                vs = sb.tile([128, T], F32, name=f"vec{v}")
                nc.vector.tensor_copy(vs[:], pv[:])
                vec_sb.append(vs)

            def flat(ap):
                return ap[:]

            # ---------------- i/o/u gates ----------------
            gates = []
            for v, (g_nm, b_nm, fn, nm) in enumerate([
                    ("gi", "bi", AF.Sigmoid, "ig"),
                    ("go", "bo", AF.Sigmoid, "og"),
                    ("gu", "bu", AF.Tanh, "ug")]):
                lnv = sb.tile([128, T], F32, name=nm + "_ln")
                nc.gpsimd.layernorm(lnv[:], vec_sb[v][:],
                                    gamma_ap=ln_t[g_nm], beta_ap=ln_t[b_nm],
                                    eps=EPS, subtract_mean=True)
                out = sb.tile([128, T], F32, name=nm)
                nc.scalar.activation(out[:], lnv[:], fn)
                gates.append(out)
            i_sb, o_sb, u_sb = gates

            cell_lin = sb.tile([128, T], F32, name="cell_lin")
            nc.vector.tensor_tensor(cell_lin[:], i_sb[:], u_sb[:],
                                    op=ALU.mult)
            nc.vector.tensor_tensor(cell_lin[:], cell_lin[:],
                                    vec_sb[3][:], op=ALU.add)
            new_c = sb.tile([128, T], F32, name="new_c")
            nc.gpsimd.layernorm(new_c[:], cell_lin[:],
                                gamma_ap=ln_t["gc"], beta_ap=ln_t["bc"],
                                eps=EPS, subtract_mean=True)
            th = sb.tile([128, T], F32, name="th")
            nc.scalar.activation(th[:], new_c[:], AF.Tanh)
            new_h = sb.tile([128, T], F32, name="new_h")
            nc.vector.tensor_tensor(new_h[:], o_sb[:], th[:], op=ALU.mult)

            nc.sync.dma_start(out_c[:], new_c[:])
            nc.scalar.dma_start(out_h[:], new_h[:])

            if dbg:
                for nm, src in [("d_lg", lg_pad[0:N, :]), ("d_exps", exps_row),
                                ("d_ml", ml_sb), ("d_mh", mh_pre),
                                ("d_frstd", frstd), ("d_f", f_sb),
                                ("d_fc", fc_sb), ("d_iou", iou_sb),
                                ("d_lni", gates[0]), ("d_cl", cell_lin)]:
                    dd = sb.tile(list(dbg_t[nm].shape), F32, name=nm + "_d")
                    nc.vector.tensor_copy(dd[:], src[:])
                    nc.sync.dma_start(dbg_t[nm][:], dd[:])
                dmh2 = sb.tile([128, T], F32, name="d_mhbf_d")
                nc.vector.tensor_copy(dmh2[:], mh_bf[:])
                nc.sync.dma_start(dbg_t["d_mhbf"][:], dmh2[:])
                for nm, src in [("d_vec0", vec_sb[0]), ("d_vec3", vec_sb[3])]:
                    dd = sb.tile([128, 16], F32, name=nm + "_d")
                    nc.vector.tensor_copy(dd[:], src[:])
                    nc.sync.dma_start(dbg_t[nm][:], dd[:])

    nc.compile()
    return nc


def _tmaj(v):
    """[2048] vector -> [128,16] t-major sbuf image (sb[p,t] = v[t*128+p])."""
    return np.ascontiguousarray(v.reshape(T, 128).T)


def _scmaj(v):
    """[2048] vector -> [128,16] sc-major image (sb[p,s*8+c] = v[c*256+s*128+p])."""
    return np.ascontiguousarray(
        v.reshape(8, 2, 128).transpose(2, 1, 0).reshape(128, 16))


def _ktiles(wT, cols):
    """wT: [K_in, out_cols] -> [128, (K_in/128)*cols] partition-major pack."""
    k_in = wT.shape[0]
    return np.ascontiguousarray(
        wT.reshape(k_in // 128, 128, cols).transpose(1, 0, 2).reshape(
            128, (k_in // 128) * cols))


def kernel(input, hiddens, cells, external,
           W_ai, W_attn, W_merge, W_iou, W_fi, W_fh,
           g_merge, b_merge, g_f, b_f, g_i, b_i, g_o, b_o, g_u, b_u,
           g_c, b_c, _dbg=False):
    key = ("nc", _dbg)
    if key not in _CACHE:
        _CACHE[key] = _build(_dbg)
    nc = _CACHE[key]

    f32 = np.float32
    input = np.asarray(input, f32)
    hiddens = np.asarray(hiddens, f32)
    cells = np.asarray(cells, f32)
    external = np.asarray(external, f32)

    hTt = _ktiles(np.ascontiguousarray(hiddens.T), N)
    xT32 = _ktiles(np.tile(input[:, None], (1, N)), N)
    eT32 = _ktiles(np.tile(external[:, None], (1, N)), N)
    x1 = _tmaj(input)

    cpack128 = np.zeros((128, CP128), f32)
    for i, v in enumerate([g_merge, b_merge, g_i, b_i, g_o, b_o,
                           g_u, b_u, g_c, b_c]):
        cpack128[:, C_LN0 + i * T:C_LN0 + (i + 1) * T] = _tmaj(
            np.asarray(v, f32))
    cpack128[:, C_ONE] = 1.0
    cpack128[0, C_ONESR:C_ONESR + 128] = 1.0
    cpack128[:, C_MAGIC] = np.full(128, MAGIC, np.int32).view(f32)
    for c in range(8):
        for h2 in range(2):
            cpack128[c, C_SEL + h2 * T + 2 * c + h2] = 1.0

    Wf_cat = np.concatenate([W_fh, W_fi], axis=1)              # [H, 4096]
    in_maps = []
    for c in range(NC):
        r = slice(c * S, (c + 1) * S)
        iou_rows = np.concatenate(
            [W_iou[g * H + c * S:g * H + (c + 1) * S, :] for g in range(3)],
            axis=0)                                            # [768, 4096]
        wiou_full = _ktiles(np.ascontiguousarray(iou_rows.T), 3 * S)
        actpack = np.concatenate([
            hTt, xT32, eT32, x1,
            np.ascontiguousarray(
                hiddens.T[c * S:(c + 1) * S].reshape(2, 128, N)
                .transpose(1, 0, 2).reshape(128, 2 * N)),
        ], axis=1).astype(NPBF)
        cpack32 = np.concatenate([
            np.ascontiguousarray(cells[:, r]),
            np.tile(np.asarray(g_f, f32)[r], (N, 1)),
            np.tile(np.asarray(b_f, f32)[r], (N, 1)),
            np.tile(np.asarray(W_attn, f32)[0, r], (N, 1)),
        ], axis=1).astype(f32)
        m = {
            "actpack": np.ascontiguousarray(actpack),
            "x1d": np.ascontiguousarray(x1.astype(NPBF)),
            "cpack32": np.ascontiguousarray(cpack32),
            "cpack128": cpack128,
            "wai": _ktiles(np.ascontiguousarray(W_ai[r].T), S).astype(NPBF),
            "wf": _ktiles(np.ascontiguousarray(Wf_cat[r].T), S).astype(NPBF),
            "wmg": _ktiles(np.ascontiguousarray(W_merge[:, r].T),
                           H).astype(NPBF),
            "wioux": np.ascontiguousarray(
                wiou_full[:, :T * 3 * S]).astype(NPBF),
            "wioumh": np.ascontiguousarray(
                wiou_full[:, T * 3 * S:]).astype(NPBF),
        }
        in_maps.append(m)

    res = run_bass_kernel_spmd(nc, in_maps, core_ids=list(range(NC)))
    _CACHE["last_results"] = res
    r0 = res.results[0]
    new_h = np.ascontiguousarray(r0["out_h"].T).reshape(H)
    new_c = np.ascontiguousarray(r0["out_c"].T).reshape(H)
    return new_h, new_c
